# revision 1
# baseline (speedup 1.0000x reference)
"""Trainium2 Bass kernel for nn_LCN (locally-connected network).

Computation (see module docstring math):
  x: (512, 1, 280, 280) -> non-overlapping 28x28 patches (10x10 grid, P=100)
  y[b, f, p] = sum_{k,l} x[b, 28ph+k, 28pw+l] * w[f*100+p, 0, k, l]
  y = relu(y + bias[f*100+p]);  out = y_flat @ dec_w.T + dec_b   (j = f*100 + p)

Sharding: 8 cores = 4 batch groups x 2 image halves (rows 0..139 | 140..279).
Each core: 128 images, 5 bands (28 rows each), 50 patches.
Per core pipeline:
  - DMA band [128b, 7840] (fp32, contiguous in HBM)
  - PE transpose x chunks [128b, 112pix] -> PSUM [112, 128] (identity matmul)
  - DVE/ACT evacuate PSUM -> SBUF
  - per patch: 7 accumulating matmuls lhsT=w[112,16], rhs=xT[112,128] -> y PSUM
    (4 patches per PSUM tile at partition offsets 0/32/64/96)
  - ACT: relu(y + bias) -> y_sb  [j on partitions, gappy layout]
  - decoder: 13 accumulating matmuls lhsT=dec[128,10] (zeros in gaps) -> out [10,128]
Host sums the two half-image partial decoder outputs and adds dec_b.
"""

import sys

import numpy as np

for _p in ("/opt/trn_rl_repo", "/opt/trn_rl_repo/concourse"):
    if _p not in sys.path:
        sys.path.insert(0, _p)

import concourse.bass as bass
import concourse.mybir as mybir
import concourse.tile as tile
from concourse import bacc
from concourse.masks import make_identity

F32 = mybir.dt.float32

# Problem constants
B, H, W = 512, 280, 280
KS = 28
HS = WS = 10
F = 16
OUT = 10
NCORES = 8
BLOC = 128      # images per core
NBANDS = 5      # bands per core (half image)
NPW = 10        # patches per band
NCHUNK = 7      # 112-pixel chunks per patch (4 rows x 28 cols each)
CK = 112        # contraction chunk size
BAND_W = KS * W  # 7840 elements per band per image


def build_program(n_bands=NBANDS, n_pw=NPW, use_is_transpose=True, use_bf16=False):
    np_loc = n_bands * n_pw
    ng = (np_loc + 3) // 4
    WDT = mybir.dt.bfloat16 if use_bf16 else F32

    nc = bacc.Bacc("TRN2")
    x_d = nc.dram_tensor("x", [BLOC, n_bands * BAND_W], F32, kind="ExternalInput")
    w_d = nc.dram_tensor("w", [CK, np_loc * NCHUNK * F], WDT, kind="ExternalInput")
    b_d = nc.dram_tensor("bias", [128, ng], F32, kind="ExternalInput")
    d_d = nc.dram_tensor("dec", [128, ng * OUT], F32, kind="ExternalInput")
    o_d = nc.dram_tensor("out", [OUT, BLOC], F32, kind="ExternalOutput")

    with tile.TileContext(nc) as tc:
        with (
            tc.tile_pool(name="const", bufs=1) as constp,
            tc.tile_pool(name="xb", bufs=2) as xbp,
            tc.tile_pool(name="xpm", bufs=2) as xpmp,
            tc.tile_pool(name="xt", bufs=3) as xtp,
            tc.tile_pool(name="xtps", bufs=2, space="PSUM") as xtpsp,
            tc.tile_pool(name="yps", bufs=2, space="PSUM") as ypsp,
            tc.tile_pool(name="ops", bufs=1, space="PSUM") as opsp,
        ):
            ident = constp.tile([128, 128], F32)
            make_identity(nc, ident[:])
            zero_sb = constp.tile([128, 128], F32)
            nc.gpsimd.memset(zero_sb[:], 0.0)
            w_sb = constp.tile([CK, np_loc * NCHUNK * F], WDT)
            nc.sync.dma_start(out=w_sb[:], in_=w_d[:])
            bias_sb = constp.tile([128, ng], F32)
            nc.sync.dma_start(out=bias_sb[:], in_=b_d[:])
            dec_sb = constp.tile([128, ng * OUT], F32)
            nc.sync.dma_start(out=dec_sb[:], in_=d_d[:])
            y_sb = constp.tile([128, ng * 128], F32)

            x_tiles = {}
            xpm_tiles = {}

            def load_band(b):
                t = xbp.tile([128, BAND_W], F32, name="x_sb")
                nc.sync.dma_start(out=t[:], in_=x_d[:, b * BAND_W:(b + 1) * BAND_W])
                x_tiles[b] = t

            def im2col(b):
                # reorder band [b, (k pw l)] -> patch-major [b, (pw k l)]
                # so transpose lhsT chunks are contiguous (walrus: 1 free dim)
                t = xpmp.tile([128, BAND_W], F32, name="x_pm")
                src = x_tiles[b][:].rearrange(
                    "b (k pw l) -> b pw k l", k=KS, pw=NPW)
                if b % 2 == 0:
                    nc.vector.tensor_copy(t[:], src)
                else:
                    nc.scalar.activation(
                        out=t[:], in_=src,
                        func=mybir.ActivationFunctionType.Copy)
                xpm_tiles[b] = t
                x_tiles.pop(b)

            y_tiles = {}

            def emit_mms(pp, xtA, xtB):
                G, q = pp // 4, pp % 4
                if G not in y_tiles:
                    yt = ypsp.tile([128, 128], F32, name="y_ps")
                    if G < 2:
                        # clear stale/NaN PSUM so gap partitions are finite
                        nc.vector.tensor_copy(yt[:], zero_sb[:])
                    y_tiles[G] = yt
                yt = y_tiles[G]
                for t in range(NCHUNK):
                    if t < 4:
                        rhs = xtA[:, t * 128:(t + 1) * 128]
                    else:
                        rhs = xtB[:, (t - 4) * 128:(t - 3) * 128]
                    nc.tensor.matmul(
                        yt[32 * q:32 * q + F, :],
                        w_sb[:, (pp * NCHUNK + t) * F:(pp * NCHUNK + t + 1) * F],
                        rhs,
                        start=(t == 0),
                        stop=(t == NCHUNK - 1),
                        tile_position=(0, 32 * q),
                    )
                if q == 3 or pp == np_loc - 1:
                    nc.scalar.activation(
                        out=y_sb[:, G * 128:(G + 1) * 128],
                        in_=yt[:],
                        func=mybir.ActivationFunctionType.Relu,
                        bias=bias_sb[:, G:G + 1],
                    )

            prev = None
            for p in range(np_loc):
                band, pw = p // n_pw, p % n_pw
                if pw == 0:
                    if band == 0:
                        load_band(0)
                        if n_bands > 1:
                            load_band(1)
                        im2col(0)
                    if band + 1 < n_bands:
                        if band + 2 < n_bands:
                            load_band(band + 2)
                        im2col(band + 1)
                x_pm = xpm_tiles[band]
                xtA_ps = xtpsp.tile([CK, 512], F32, name="xtA_ps")
                xtB_ps = xtpsp.tile([CK, 384], F32, name="xtB_ps")
                for t in range(NCHUNK):
                    if t < 4:
                        dst = xtA_ps[:, t * 128:(t + 1) * 128]
                    else:
                        dst = xtB_ps[:, (t - 4) * 128:(t - 3) * 128]
                    src = x_pm[:, pw * 784 + t * CK: pw * 784 + (t + 1) * CK]
                    if use_is_transpose:
                        nc.tensor.transpose(dst, src, ident[:])
                    else:
                        nc.tensor.matmul(dst, src, ident[:])
                xtA = xtp.tile([CK, 512], WDT, name="xtA")
                xtB = xtp.tile([CK, 384], WDT, name="xtB")
                if p % 2 == 0:
                    nc.vector.tensor_copy(xtA[:], xtA_ps[:])
                    nc.vector.tensor_copy(xtB[:], xtB_ps[:])
                else:
                    nc.scalar.activation(
                        out=xtA[:], in_=xtA_ps[:],
                        func=mybir.ActivationFunctionType.Copy)
                    nc.scalar.activation(
                        out=xtB[:], in_=xtB_ps[:],
                        func=mybir.ActivationFunctionType.Copy)
                if prev is not None:
                    emit_mms(*prev)
                prev = (p, xtA, xtB)
            emit_mms(*prev)

            # stage 2: decoder  out[o, b] = sum_j dec[j, o] * y[j, b]
            out_ps = opsp.tile([OUT, BLOC], F32)
            for G in range(ng):
                nc.tensor.matmul(
                    out_ps[:],
                    dec_sb[:, G * OUT:(G + 1) * OUT],
                    y_sb[:, G * 128:(G + 1) * 128],
                    start=(G == 0),
                    stop=(G == ng - 1),
                )
            out_sb = constp.tile([OUT, BLOC], F32)
            nc.vector.tensor_copy(out_sb[:], out_ps[:])
            nc.sync.dma_start(out=o_d[:], in_=out_sb[:])

    return nc


def stage_half(weight, bias, dec_w, h, n_bands=NBANDS, n_pw=NPW):
    """Host-side staging of weights/bias/decoder for image-half h (0 or 1)."""
    np_loc = n_bands * n_pw
    ng = (np_loc + 3) // 4
    weight = np.asarray(weight, np.float32)
    bias = np.asarray(bias, np.float32)
    dec_w = np.asarray(dec_w, np.float32)

    # w: (1600, 1, 28, 28) -> [f, ph, pw, k, l] -> chunks [d=(kk,l), (bl,pw,t,f)]
    w5 = weight.reshape(F, HS, WS, KS, KS)[:, n_bands * h:n_bands * h + n_bands]
    w6 = w5.reshape(F, n_bands, WS, NCHUNK, 4, KS)  # f bl pw t kk l
    wst = np.ascontiguousarray(
        np.transpose(w6, (4, 5, 1, 2, 3, 0))).reshape(CK, np_loc * NCHUNK * F)

    b5 = bias.reshape(F, HS, WS)[:, n_bands * h:n_bands * h + n_bands, :]
    b5 = b5.reshape(F, np_loc)
    bst = np.zeros((128, ng), np.float32)
    d5 = dec_w.reshape(OUT, F, HS, WS)[:, :, n_bands * h:n_bands * h + n_bands, :]
    d5 = d5.reshape(OUT, F, np_loc)
    dst_ = np.zeros((128, ng * OUT), np.float32)
    for pl in range(np_loc):
        G, q = pl // 4, pl % 4
        bst[32 * q:32 * q + F, G] = b5[:, pl]
        dst_[32 * q:32 * q + F, G * OUT:(G + 1) * OUT] = d5[:, :, pl].T
    return wst, bst, dst_


_cache = {}
USE_BF16 = False
USE_IS_TRANSPOSE = True


def _get_nc():
    key = ("nc", USE_BF16, USE_IS_TRANSPOSE)
    if key not in _cache:
        nc = build_program(use_is_transpose=USE_IS_TRANSPOSE, use_bf16=USE_BF16)
        nc.finalize()
        _cache[key] = nc
    return _cache[key]


def make_in_maps(x, weight, bias, dec_w):
    x = np.asarray(x, np.float32)
    stages = [stage_half(weight, bias, dec_w, h) for h in (0, 1)]
    in_maps = []
    for core in range(NCORES):
        bg, h = core // 2, core % 2
        xs = np.ascontiguousarray(
            x[bg * BLOC:(bg + 1) * BLOC, 0, 140 * h:140 * h + 140, :]
        ).reshape(BLOC, NBANDS * BAND_W)
        wst, bst, dst_ = stages[h]
        if USE_BF16:
            import ml_dtypes
            wst = wst.astype(ml_dtypes.bfloat16)
        in_maps.append({"x": xs, "w": wst, "bias": bst, "dec": dst_})
    return in_maps


def combine(results, dec_b):
    out = np.zeros((B, OUT), np.float32)
    for bg in range(4):
        part = results[2 * bg]["out"] + results[2 * bg + 1]["out"]  # (10, 128)
        out[bg * BLOC:(bg + 1) * BLOC] = part.T + np.asarray(dec_b, np.float32)
    return out


def _install_ntff_hook():
    """Provide the missing antenv.axon_hooks module so trace=True works
    under axon (replicates trn_boot._ntff_profile_via_ctypes)."""
    import contextlib
    import ctypes
    import types

    if "antenv.axon_hooks" in sys.modules:
        return
    so_path = "/opt/axon/libaxon_pjrt.so"
    holder = {}
    mod = types.ModuleType("antenv.axon_hooks")
    mod.set_axon_ntff_profile_hook = lambda h: holder.__setitem__("h", h)
    mod.get_axon_ntff_profile_hook = lambda: holder.get("h")
    sys.modules["antenv.axon_hooks"] = mod
    try:
        import antenv
        antenv.axon_hooks = mod
    except ImportError:
        pass

    lib = ctypes.CDLL(so_path)
    if not hasattr(lib, "axon_start_nrt_profile"):
        return
    lib.axon_start_nrt_profile.argtypes = [
        ctypes.POINTER(ctypes.c_int64), ctypes.c_size_t]
    lib.axon_start_nrt_profile.restype = ctypes.c_int64
    lib.axon_stop_nrt_profile.argtypes = [ctypes.c_char_p]
    lib.axon_stop_nrt_profile.restype = ctypes.c_int64

    @contextlib.contextmanager
    def _hook(output_dir, device_ids):
        import jax
        jax.devices()
        if device_ids:
            ids = (ctypes.c_int64 * len(device_ids))(*device_ids)
            rc = lib.axon_start_nrt_profile(ids, len(device_ids))
        else:
            rc = lib.axon_start_nrt_profile(None, 0)
        if rc != 0:
            raise RuntimeError(f"axon_start_nrt_profile rc={rc}")
        try:
            yield
        finally:
            n = lib.axon_stop_nrt_profile(str(output_dir).encode())
            print(f"profile: {n} file(s) written to {output_dir}")

    mod.set_axon_ntff_profile_hook(_hook)


def run(x, weight, bias, dec_w, dec_b, trace=False):
    from concourse import bass_utils
    from concourse.bass_utils import run_bass_kernel_spmd

    if trace:
        _install_ntff_hook()
        # artifact upload needs a bucket that doesn't exist here
        bass_utils.upload_artifacts = lambda tmpdir: tmpdir

    nc = _get_nc()
    in_maps = make_in_maps(x, weight, bias, dec_w)
    r = run_bass_kernel_spmd(nc, in_maps, list(range(NCORES)), trace=trace)
    return combine(r.results, dec_b), r


def kernel(x, weight, bias, dec_w, dec_b):
    out, _ = run(x, weight, bias, dec_w, dec_b, trace=False)
    return out



# revision 3
# speedup vs baseline: 3.1223x; 3.1223x over previous
"""Trainium2 Bass kernel for nn_LCN (locally-connected network).

Computation:
  x: (512, 1, 280, 280) -> non-overlapping 28x28 patches (10x10 grid, P=100)
  y[b, f, p] = sum_q x[b, p, q] * w[f*100+p, q]    (q = k*28+l, 784 per patch)
  y = relu(y + bias[f*100+p]);  out = y_flat @ dec_w.T + dec_b  (j = f*100+p)

Sharding: patch-parallel. All 8 cores see all 512 images; core c owns 13
patches (cores 4-7 own 12 real + 1 zero patch so every core runs the same
program). Per core:
  - host stages x as [512, 10240] bf16, patch-major pixels (im2col on host)
  - 5 xbar DMA-transposes load it as xT [128 px, 80 chunks, 512 b] in SBUF
  - conv: one matmul per (128-px chunk, patch-pair window): lhsT = staged
    weight tile [128, 32], rhs = xT[:, t, :] [128, 512] -> PSUM [128, 512]
    accumulating per 8-patch group (partition j = 16*local_patch + f)
  - ACT: relu(psum + bias) -> y_sb
  - decoder: 2 accumulating matmuls lhsT=dec [K, 10] -> out [10, 512]
Host sums the 8 per-core partial decoder outputs and adds dec_b.
"""

import sys

import numpy as np

for _p in ("/opt/trn_rl_repo", "/opt/trn_rl_repo/concourse"):
    if _p not in sys.path:
        sys.path.insert(0, _p)

import concourse.bass as bass
import concourse.mybir as mybir
import concourse.tile as tile
from concourse import bacc

F32 = mybir.dt.float32
BF16 = mybir.dt.bfloat16

# Problem constants
B = 512
P = 100
F = 16
OUT = 10
PPX = 784            # pixels per patch (28*28)
NCORES = 8

NPAT = 13            # patches per core program (zero-padded on 12-patch cores)
NCHUNK = 80          # ceil(13*784/128)
PXPAD = NCHUNK * 128  # 10240
GROUPS = [(0, 8), (8, 5)]   # (first local patch, n patches incl virtual pad)
GROUP_CHUNKS = [(0, 49), (49, 31)]  # (first chunk, n chunks); 8*784 = 49*128
DMA_SPLITS = [(0, 16), (16, 16), (32, 16), (48, 16), (64, 16)]

# per-core real patch ranges (cores 0-3: 13 patches, cores 4-7: 12)
CORE_PSTART = [0, 13, 26, 39, 52, 64, 76, 88]
CORE_NPAT = [13, 13, 13, 13, 12, 12, 12, 12]


def conv_plan():
    """Static matmul plan: one entry per (chunk, patch-pair window)."""
    plan = []
    for t in range(NCHUNK):
        p0 = (128 * t) // PPX
        p1 = (128 * t + 127) // PPX
        g = p0 // 8
        pairs = sorted({(min(p, 8 * g + 9) - 8 * g) // 2 for p in (p0, p1)})
        for k in pairs:
            first = ((8 * g + 2 * k) * PPX) // 128
            last = min(((8 * g + 2 * k + 2) * PPX - 1) // 128, NCHUNK - 1)
            plan.append((t, g, k, t == first, t == last))
    return plan

PLAN = conv_plan()
NMM = len(PLAN)  # 85


def build_program():
    nc = bacc.Bacc("TRN2")
    x_d = nc.dram_tensor("x", [B, PXPAD], BF16, kind="ExternalInput")
    w_d = nc.dram_tensor("w", [128, NMM * 32], BF16, kind="ExternalInput")
    b_d = nc.dram_tensor("bias", [128, 2], F32, kind="ExternalInput")
    d_d = nc.dram_tensor("dec", [128, 2 * OUT], F32, kind="ExternalInput")
    o_d = nc.dram_tensor("out", [OUT, B], F32, kind="ExternalOutput")

    with tile.TileContext(nc) as tc:
        with (
            tc.tile_pool(name="const", bufs=1) as constp,
            tc.tile_pool(name="yps", bufs=2, space="PSUM") as ypsp,
            tc.tile_pool(name="ops", bufs=1, space="PSUM") as opsp,
        ):
            w_sb = constp.tile([128, NMM * 32], BF16)
            nc.sync.dma_start(out=w_sb[:], in_=w_d[:])
            bias_sb = constp.tile([128, 2], F32)
            nc.sync.dma_start(out=bias_sb[:], in_=b_d[:])
            dec_sb = constp.tile([128, 2 * OUT], F32)
            nc.sync.dma_start(out=dec_sb[:], in_=d_d[:])

            xt = constp.tile([128, NCHUNK, B], BF16)
            for c0, n in DMA_SPLITS:
                nc.sync.dma_start(
                    out=xt[:, c0:c0 + n, :],
                    in_=x_d[:, c0 * 128:(c0 + n) * 128],
                    transpose=True,
                )

            y_sb = constp.tile([128, 2, B], F32)
            ps = [ypsp.tile([128, B], F32, name=f"ps{g}") for g in range(2)]

            rows = [16 * 8, 16 * 5]  # evacuated rows per group
            for i, (t, g, k, st, sp) in enumerate(PLAN):
                nc.tensor.matmul(
                    ps[g][32 * k:32 * k + 32, :],
                    w_sb[:, 32 * i:32 * i + 32],
                    xt[:, t, :],
                    start=st,
                    stop=sp,
                    tile_position=(0, 32 * k),
                )
            for g in range(2):
                nc.scalar.activation(
                    out=y_sb[0:rows[g], g, :],
                    in_=ps[g][0:rows[g], :],
                    func=mybir.ActivationFunctionType.Relu,
                    bias=bias_sb[0:rows[g], g:g + 1],
                )

            out_ps = opsp.tile([OUT, B], F32)
            for g in range(2):
                nc.tensor.matmul(
                    out_ps[:],
                    dec_sb[0:rows[g], g * OUT:(g + 1) * OUT],
                    y_sb[0:rows[g], g, :],
                    start=(g == 0),
                    stop=(g == 1),
                )
            out_sb = constp.tile([OUT, B], F32)
            nc.vector.tensor_copy(out_sb[:], out_ps[:])
            nc.sync.dma_start(out=o_d[:], in_=out_sb[:])

    return nc


def stage_core(core, x_pm, weight, bias, dec_w):
    """Host-side staging for one core. x_pm: (B, 100, 784) float32."""
    import ml_dtypes

    p0 = CORE_PSTART[core]
    npr = CORE_NPAT[core]
    pids = list(range(p0, p0 + npr))

    xs = np.zeros((B, PXPAD), np.float32)
    xs[:, :npr * PPX] = x_pm[:, p0:p0 + npr, :].reshape(B, npr * PPX)
    xs = xs.astype(ml_dtypes.bfloat16)

    wr = np.asarray(weight, np.float32).reshape(F, P, PPX)
    w_big = np.zeros((128, NMM * 32), np.float32)
    for i, (t, g, k, _, _) in enumerate(PLAN):
        for r in range(128):
            px = 128 * t + r
            p = px // PPX
            if p >= npr:
                continue
            pl = p - 8 * g
            if pl < 0 or pl // 2 != k:
                continue
            q = px % PPX
            w_big[r, 32 * i + (pl % 2) * 16:32 * i + (pl % 2) * 16 + F] = \
                wr[:, pids[p], q]
    w_big = w_big.astype(ml_dtypes.bfloat16)

    br = np.asarray(bias, np.float32).reshape(F, P)
    dr = np.asarray(dec_w, np.float32).reshape(OUT, F, P)
    b_st = np.zeros((128, 2), np.float32)
    d_st = np.zeros((128, 2 * OUT), np.float32)
    for p in range(npr):
        g, pl = p // 8, p % 8
        j = 16 * pl + np.arange(F)
        b_st[j, g] = br[:, pids[p]]
        d_st[j[:, None], g * OUT + np.arange(OUT)[None, :]] = dr[:, :, pids[p]].T
    return {"x": xs, "w": w_big, "bias": b_st, "dec": d_st}


_cache = {}


def _get_nc():
    if "nc" not in _cache:
        nc = build_program()
        nc.finalize()
        _cache["nc"] = nc
    return _cache["nc"]


def make_in_maps(x, weight, bias, dec_w):
    x = np.asarray(x, np.float32)
    # patch-major pixel order: (b, ph, pw, k, l)
    x_pm = np.ascontiguousarray(
        x.reshape(B, 10, 28, 10, 28).transpose(0, 1, 3, 2, 4)
    ).reshape(B, P, PPX)
    return [stage_core(c, x_pm, weight, bias, dec_w) for c in range(NCORES)]


def combine(results, dec_b):
    acc = np.zeros((OUT, B), np.float32)
    for r in results:
        acc += r["out"]
    return acc.T + np.asarray(dec_b, np.float32)


def _install_ntff_hook():
    """Provide the missing antenv.axon_hooks module so trace=True works
    under axon (replicates trn_boot._ntff_profile_via_ctypes)."""
    import contextlib
    import ctypes
    import types

    if "antenv.axon_hooks" in sys.modules:
        return
    so_path = "/opt/axon/libaxon_pjrt.so"
    holder = {}
    mod = types.ModuleType("antenv.axon_hooks")
    mod.set_axon_ntff_profile_hook = lambda h: holder.__setitem__("h", h)
    mod.get_axon_ntff_profile_hook = lambda: holder.get("h")
    sys.modules["antenv.axon_hooks"] = mod
    try:
        import antenv
        antenv.axon_hooks = mod
    except ImportError:
        pass

    lib = ctypes.CDLL(so_path)
    if not hasattr(lib, "axon_start_nrt_profile"):
        return
    lib.axon_start_nrt_profile.argtypes = [
        ctypes.POINTER(ctypes.c_int64), ctypes.c_size_t]
    lib.axon_start_nrt_profile.restype = ctypes.c_int64
    lib.axon_stop_nrt_profile.argtypes = [ctypes.c_char_p]
    lib.axon_stop_nrt_profile.restype = ctypes.c_int64

    @contextlib.contextmanager
    def _hook(output_dir, device_ids):
        import jax
        jax.devices()
        if device_ids:
            ids = (ctypes.c_int64 * len(device_ids))(*device_ids)
            rc = lib.axon_start_nrt_profile(ids, len(device_ids))
        else:
            rc = lib.axon_start_nrt_profile(None, 0)
        if rc != 0:
            raise RuntimeError(f"axon_start_nrt_profile rc={rc}")
        try:
            yield
        finally:
            n = lib.axon_stop_nrt_profile(str(output_dir).encode())
            print(f"profile: {n} file(s) written to {output_dir}")

    mod.set_axon_ntff_profile_hook(_hook)


def run(x, weight, bias, dec_w, dec_b, trace=False):
    from concourse import bass_utils
    from concourse.bass_utils import run_bass_kernel_spmd

    if trace:
        _install_ntff_hook()
        bass_utils.upload_artifacts = lambda tmpdir: tmpdir

    nc = _get_nc()
    in_maps = make_in_maps(x, weight, bias, dec_w)
    r = run_bass_kernel_spmd(nc, in_maps, list(range(NCORES)), trace=trace)
    return combine(r.results, dec_b), r


def kernel(x, weight, bias, dec_w, dec_b):
    out, _ = run(x, weight, bias, dec_w, dec_b, trace=False)
    return out


# revision 7
# speedup vs baseline: 3.9746x; 1.2730x over previous
"""Trainium2 Bass kernel for nn_LCN (locally-connected network).

Computation:
  x: (512, 1, 280, 280) -> non-overlapping 28x28 patches (10x10 grid, P=100)
  y[b, f, p] = sum_q x[b, p, q] * w[f*100+p, q]    (q = k*28+l, 784 per patch)
  y = relu(y + bias[f*100+p]);  out = y_flat @ dec_w.T + dec_b  (j = f*100+p)

Sharding: patch-parallel. All 8 cores see all 512 images; core c owns 13
patches (cores 4-7 own 12 real + 1 zero patch so every core runs the same
program). Per core:
  - host stages x TRANSPOSED as xT [128 px, 80 chunks, 512 b] bf16
    (im2col + transpose + cast all on host; DMA reads are contiguous
    multi-KB runs per partition at full HBM bandwidth)
  - conv: one matmul per (128-px chunk, patch-pair window): lhsT = staged
    weight tile [128, 32], rhs = xT[:, t, :] [128, 512] -> PSUM [128, 512]
    accumulating per 8-patch group (partition j = 16*local_patch + f)
  - ACT: relu(psum + bias) -> y_sb
  - decoder: 2 accumulating matmuls lhsT=dec [K, 10] -> out [10, 512]
Host sums the 8 per-core partial decoder outputs and adds dec_b.
"""

import sys

import numpy as np

for _p in ("/opt/trn_rl_repo", "/opt/trn_rl_repo/concourse"):
    if _p not in sys.path:
        sys.path.insert(0, _p)

import concourse.bass as bass
import concourse.mybir as mybir
import concourse.tile as tile
from concourse import bacc

F32 = mybir.dt.float32
BF16 = mybir.dt.bfloat16

# Problem constants
B = 512
P = 100
F = 16
OUT = 10
PPX = 784            # pixels per patch (28*28)
NCORES = 8

NPAT = 13            # patches per core program (zero-padded on 12-patch cores)
NCHUNK = 80          # ceil(13*784/128)
PXPAD = NCHUNK * 128  # 10240
GROUPS = [(0, 8), (8, 5)]   # (first local patch, n patches incl virtual pad)
GROUP_CHUNKS = [(0, 49), (49, 31)]  # (first chunk, n chunks); 8*784 = 49*128
# small first split to warm up PE early, small last split to trim the tail
_SPLIT_SIZES = [6, 10, 12, 12, 12, 12, 12, 4]
DMA_SPLITS = []
_c = 0
for _s in _SPLIT_SIZES:
    DMA_SPLITS.append((_c, _s))
    _c += _s
assert _c == NCHUNK

# per-core real patch ranges (cores 0-3: 13 patches, cores 4-7: 12)
CORE_PSTART = [0, 13, 26, 39, 52, 64, 76, 88]
CORE_NPAT = [13, 13, 13, 13, 12, 12, 12, 12]


def conv_plan():
    """Static matmul plan: one entry per (chunk, patch-pair window)."""
    plan = []
    for t in range(NCHUNK):
        p0 = (128 * t) // PPX
        p1 = (128 * t + 127) // PPX
        g = p0 // 8
        pairs = sorted({(min(p, 8 * g + 9) - 8 * g) // 2 for p in (p0, p1)})
        for k in pairs:
            first = ((8 * g + 2 * k) * PPX) // 128
            last = min(((8 * g + 2 * k + 2) * PPX - 1) // 128, NCHUNK - 1)
            plan.append((t, g, k, t == first, t == last))
    return plan

PLAN = conv_plan()
NMM = len(PLAN)  # 85


def build_program():
    nc = bacc.Bacc("TRN2")
    x_d = nc.dram_tensor("x", [128, NCHUNK * B], BF16, kind="ExternalInput")
    w_d = nc.dram_tensor("w", [128, NMM * 32], BF16, kind="ExternalInput")
    b_d = nc.dram_tensor("bias", [128, 2], F32, kind="ExternalInput")
    d_d = nc.dram_tensor("dec", [128, 2 * OUT], F32, kind="ExternalInput")
    o_d = nc.dram_tensor("out", [OUT, B], F32, kind="ExternalOutput")

    with tile.TileContext(nc) as tc:
        with (
            tc.tile_pool(name="const", bufs=1) as constp,
            tc.tile_pool(name="yps", bufs=2, space="PSUM") as ypsp,
            tc.tile_pool(name="ops", bufs=1, space="PSUM") as opsp,
        ):
            w_sb = constp.tile([128, NMM * 32], BF16)
            nc.gpsimd.dma_start(out=w_sb[:], in_=w_d[:])
            bias_sb = constp.tile([128, 2], F32)
            nc.gpsimd.dma_start(out=bias_sb[:], in_=b_d[:])
            dec_sb = constp.tile([128, 2 * OUT], F32)
            nc.gpsimd.dma_start(out=dec_sb[:], in_=d_d[:])

            xt = constp.tile([128, NCHUNK, B], BF16)
            for c0, n in DMA_SPLITS:
                nc.sync.dma_start(
                    out=xt[:, c0:c0 + n, :],
                    in_=x_d[:, c0 * B:(c0 + n) * B],
                )

            y_sb = constp.tile([128, 2, B], F32)
            ps = [ypsp.tile([128, B], F32, name=f"ps{g}") for g in range(2)]

            rows = [16 * 8, 16 * 5]  # evacuated rows per group
            for i, (t, g, k, st, sp) in enumerate(PLAN):
                nc.tensor.matmul(
                    ps[g][32 * k:32 * k + 32, :],
                    w_sb[:, 32 * i:32 * i + 32],
                    xt[:, t, :],
                    start=st,
                    stop=sp,
                    tile_position=(0, 32 * k),
                )
            for g in range(2):
                nc.scalar.activation(
                    out=y_sb[0:rows[g], g, :],
                    in_=ps[g][0:rows[g], :],
                    func=mybir.ActivationFunctionType.Relu,
                    bias=bias_sb[0:rows[g], g:g + 1],
                )

            out_ps = opsp.tile([OUT, B], F32)
            for g in range(2):
                nc.tensor.matmul(
                    out_ps[:],
                    dec_sb[0:rows[g], g * OUT:(g + 1) * OUT],
                    y_sb[0:rows[g], g, :],
                    start=(g == 0),
                    stop=(g == 1),
                )
            out_sb = constp.tile([OUT, B], F32)
            nc.vector.tensor_copy(out_sb[:], out_ps[:])
            nc.sync.dma_start(out=o_d[:], in_=out_sb[:])

    return nc


def stage_core(core, x_pm, weight, bias, dec_w):
    """Host-side staging for one core. x_pm: (B, 100, 784) float32."""
    import ml_dtypes

    p0 = CORE_PSTART[core]
    npr = CORE_NPAT[core]
    pids = list(range(p0, p0 + npr))

    xs = np.zeros((B, PXPAD), np.float32)
    xs[:, :npr * PPX] = x_pm[:, p0:p0 + npr, :].reshape(B, npr * PPX)
    # host-side transpose to [px_part 128, chunk, batch], bf16
    xs = np.ascontiguousarray(
        xs.reshape(B, NCHUNK, 128).transpose(2, 1, 0)
    ).astype(ml_dtypes.bfloat16).reshape(128, NCHUNK * B)

    wr = np.asarray(weight, np.float32).reshape(F, P, PPX)
    w_big = np.zeros((128, NMM * 32), np.float32)
    for i, (t, g, k, _, _) in enumerate(PLAN):
        for r in range(128):
            px = 128 * t + r
            p = px // PPX
            if p >= npr:
                continue
            pl = p - 8 * g
            if pl < 0 or pl // 2 != k:
                continue
            q = px % PPX
            w_big[r, 32 * i + (pl % 2) * 16:32 * i + (pl % 2) * 16 + F] = \
                wr[:, pids[p], q]
    w_big = w_big.astype(ml_dtypes.bfloat16)

    br = np.asarray(bias, np.float32).reshape(F, P)
    dr = np.asarray(dec_w, np.float32).reshape(OUT, F, P)
    b_st = np.zeros((128, 2), np.float32)
    d_st = np.zeros((128, 2 * OUT), np.float32)
    for p in range(npr):
        g, pl = p // 8, p % 8
        j = 16 * pl + np.arange(F)
        b_st[j, g] = br[:, pids[p]]
        d_st[j[:, None], g * OUT + np.arange(OUT)[None, :]] = dr[:, :, pids[p]].T
    return {"x": xs, "w": w_big, "bias": b_st, "dec": d_st}


_cache = {}


def _get_nc():
    if "nc" not in _cache:
        nc = build_program()
        nc.finalize()
        _cache["nc"] = nc
    return _cache["nc"]


def make_in_maps(x, weight, bias, dec_w):
    x = np.asarray(x, np.float32)
    # patch-major pixel order: (b, ph, pw, k, l)
    x_pm = np.ascontiguousarray(
        x.reshape(B, 10, 28, 10, 28).transpose(0, 1, 3, 2, 4)
    ).reshape(B, P, PPX)
    return [stage_core(c, x_pm, weight, bias, dec_w) for c in range(NCORES)]


def combine(results, dec_b):
    acc = np.zeros((OUT, B), np.float32)
    for r in results:
        acc += r["out"]
    return acc.T + np.asarray(dec_b, np.float32)


def _install_ntff_hook():
    """Provide the missing antenv.axon_hooks module so trace=True works
    under axon (replicates trn_boot._ntff_profile_via_ctypes)."""
    import contextlib
    import ctypes
    import types

    if "antenv.axon_hooks" in sys.modules:
        return
    so_path = "/opt/axon/libaxon_pjrt.so"
    holder = {}
    mod = types.ModuleType("antenv.axon_hooks")
    mod.set_axon_ntff_profile_hook = lambda h: holder.__setitem__("h", h)
    mod.get_axon_ntff_profile_hook = lambda: holder.get("h")
    sys.modules["antenv.axon_hooks"] = mod
    try:
        import antenv
        antenv.axon_hooks = mod
    except ImportError:
        pass

    lib = ctypes.CDLL(so_path)
    if not hasattr(lib, "axon_start_nrt_profile"):
        return
    lib.axon_start_nrt_profile.argtypes = [
        ctypes.POINTER(ctypes.c_int64), ctypes.c_size_t]
    lib.axon_start_nrt_profile.restype = ctypes.c_int64
    lib.axon_stop_nrt_profile.argtypes = [ctypes.c_char_p]
    lib.axon_stop_nrt_profile.restype = ctypes.c_int64

    @contextlib.contextmanager
    def _hook(output_dir, device_ids):
        import jax
        jax.devices()
        if device_ids:
            ids = (ctypes.c_int64 * len(device_ids))(*device_ids)
            rc = lib.axon_start_nrt_profile(ids, len(device_ids))
        else:
            rc = lib.axon_start_nrt_profile(None, 0)
        if rc != 0:
            raise RuntimeError(f"axon_start_nrt_profile rc={rc}")
        try:
            yield
        finally:
            n = lib.axon_stop_nrt_profile(str(output_dir).encode())
            print(f"profile: {n} file(s) written to {output_dir}")

    mod.set_axon_ntff_profile_hook(_hook)


def run(x, weight, bias, dec_w, dec_b, trace=False):
    from concourse import bass_utils
    from concourse.bass_utils import run_bass_kernel_spmd

    if trace:
        _install_ntff_hook()
        bass_utils.upload_artifacts = lambda tmpdir: tmpdir

    nc = _get_nc()
    in_maps = make_in_maps(x, weight, bias, dec_w)
    r = run_bass_kernel_spmd(nc, in_maps, list(range(NCORES)), trace=trace)
    return combine(r.results, dec_b), r


def kernel(x, weight, bias, dec_w, dec_b):
    out, _ = run(x, weight, bias, dec_w, dec_b, trace=False)
    return out


# revision 11
# speedup vs baseline: 4.1429x; 1.0424x over previous
"""Trainium2 Bass kernel for nn_LCN (locally-connected network).

Computation:
  x: (512, 1, 280, 280) -> non-overlapping 28x28 patches (10x10 grid, P=100)
  y[b, f, p] = sum_q x[b, p, q] * w[f*100+p, q]    (q = k*28+l, 784 per patch)
  y = relu(y + bias[f*100+p]);  out = y_flat @ dec_w.T + dec_b  (j = f*100+p)

Sharding: patch-parallel. All 8 cores see all 512 images; core c owns 13
patches (cores 4-7 own 12 real + 1 zero patch so every core runs the same
program). Per core:
  - host stages x TRANSPOSED as xT [128 px, 80 chunks, 512 b] bf16
    (im2col + transpose + cast all on host; DMA reads are contiguous
    multi-KB runs per partition at full HBM bandwidth)
  - conv: one matmul per (128-px chunk, patch-pair window): lhsT = staged
    weight tile [128, 32], rhs = xT[:, t, :] [128, 512] -> PSUM [128, 512]
    accumulating per 8-patch group (partition j = 16*local_patch + f)
  - ACT: relu(psum + bias) -> y_sb
  - decoder: 2 accumulating matmuls lhsT=dec [K, 10] -> out [10, 512]
Host sums the 8 per-core partial decoder outputs and adds dec_b.
"""

import sys

import numpy as np

for _p in ("/opt/trn_rl_repo", "/opt/trn_rl_repo/concourse"):
    if _p not in sys.path:
        sys.path.insert(0, _p)

import concourse.bass as bass
import concourse.mybir as mybir
import concourse.tile as tile
from concourse import bacc

F32 = mybir.dt.float32
BF16 = mybir.dt.bfloat16

# Problem constants
B = 512
P = 100
F = 16
OUT = 10
PPX = 784            # pixels per patch (28*28)
NCORES = 8

NPAT = 13            # patches per core program (zero-padded on 12-patch cores)
NCHUNK = 80          # ceil(13*784/128)
PXPAD = NCHUNK * 128  # 10240
GROUPS = [(0, 8), (8, 5)]   # (first local patch, n patches incl virtual pad)
GROUP_CHUNKS = [(0, 49), (49, 31)]  # (first chunk, n chunks); 8*784 = 49*128
# small first split to warm up PE early, small last split to trim the tail
_SPLIT_SIZES = [4, 8, 12, 12, 12, 12, 12, 8]
DMA_SPLITS = []
_c = 0
for _s in _SPLIT_SIZES:
    DMA_SPLITS.append((_c, _s))
    _c += _s
assert _c == NCHUNK

# per-core real patch ranges (cores 0-3: 13 patches, cores 4-7: 12)
CORE_PSTART = [0, 13, 26, 39, 52, 64, 76, 88]
CORE_NPAT = [13, 13, 13, 13, 12, 12, 12, 12]


def conv_plan():
    """Static matmul plan: one entry per (chunk, patch-pair window)."""
    plan = []
    for t in range(NCHUNK):
        p0 = (128 * t) // PPX
        p1 = (128 * t + 127) // PPX
        g = p0 // 8
        pairs = sorted({(min(p, 8 * g + 9) - 8 * g) // 2 for p in (p0, p1)})
        for k in pairs:
            first = ((8 * g + 2 * k) * PPX) // 128
            last = min(((8 * g + 2 * k + 2) * PPX - 1) // 128, NCHUNK - 1)
            plan.append((t, g, k, t == first, t == last))
    return plan

PLAN = conv_plan()
NMM = len(PLAN)  # 85


def build_program():
    nc = bacc.Bacc("TRN2")
    x_d = nc.dram_tensor("x", [128, NCHUNK * B], BF16, kind="ExternalInput")
    w_d = nc.dram_tensor("w", [128, NMM * 32], BF16, kind="ExternalInput")
    b_d = nc.dram_tensor("bias", [128, 2], F32, kind="ExternalInput")
    d_d = nc.dram_tensor("dec", [128, 2 * OUT], BF16, kind="ExternalInput")
    o_d = nc.dram_tensor("out", [OUT, B], F32, kind="ExternalOutput")

    with tile.TileContext(nc) as tc:
        with (
            tc.tile_pool(name="const", bufs=1) as constp,
            tc.tile_pool(name="yps", bufs=2, space="PSUM") as ypsp,
            tc.tile_pool(name="wps", bufs=1, space="PSUM") as wpsp,
            tc.tile_pool(name="ops", bufs=1, space="PSUM") as opsp,
        ):
            w_sb = constp.tile([128, NMM * 32], BF16)
            nc.sync.dma_start(out=w_sb[:], in_=w_d[:])
            bias_sb = constp.tile([128, 2], F32)
            nc.scalar.dma_start(out=bias_sb[:], in_=b_d[:])
            dec_sb = constp.tile([128, 2 * OUT], BF16)
            nc.scalar.dma_start(out=dec_sb[:], in_=d_d[:])

            xt = constp.tile([128, NCHUNK, B], BF16)
            for si, (c0, n) in enumerate(DMA_SPLITS):
                eng = nc.sync if si % 2 == 0 else nc.scalar
                eng.dma_start(
                    out=xt[:, c0:c0 + n, :],
                    in_=x_d[:, c0 * B:(c0 + n) * B],
                )

            # PE clock warm-up: M=128 dummy matmuls on already-loaded weight
            # data, during the DMA pipe-fill window (output never read)
            warm_ps = wpsp.tile([128, B], F32)
            for _ in range(10):
                nc.tensor.matmul(
                    warm_ps[:],
                    w_sb[:, 0:128],
                    w_sb[:, 0:B],
                    start=True,
                    stop=True,
                )

            y_sb = constp.tile([128, 2, B], BF16)
            ps = [ypsp.tile([128, B], F32, name=f"ps{g}") for g in range(2)]
            out_ps = opsp.tile([OUT, B], F32)

            rows = [16 * 8, 16 * 5]  # evacuated rows per group
            for g in range(2):
                gi = [e for e in enumerate(PLAN) if e[1][1] == g]
                for i, (t, _, k, st, sp) in gi:
                    nc.tensor.matmul(
                        ps[g][32 * k:32 * k + 32, :],
                        w_sb[:, 32 * i:32 * i + 32],
                        xt[:, t, :],
                        start=st,
                        stop=sp,
                        tile_position=(0, 32 * k),
                    )
                nc.scalar.activation(
                    out=y_sb[0:rows[g], g, :],
                    in_=ps[g][0:rows[g], :],
                    func=mybir.ActivationFunctionType.Relu,
                    bias=bias_sb[0:rows[g], g:g + 1],
                )
                nc.tensor.matmul(
                    out_ps[:],
                    dec_sb[0:rows[g], g * OUT:(g + 1) * OUT],
                    y_sb[0:rows[g], g, :],
                    start=(g == 0),
                    stop=(g == 1),
                )
            out_sb = constp.tile([OUT, B], F32)
            nc.vector.tensor_copy(out_sb[:], out_ps[:])
            nc.sync.dma_start(out=o_d[:], in_=out_sb[:])

    return nc


def stage_core(core, x_pm, weight, bias, dec_w):
    """Host-side staging for one core. x_pm: (B, 100, 784) float32."""
    import ml_dtypes

    p0 = CORE_PSTART[core]
    npr = CORE_NPAT[core]
    pids = list(range(p0, p0 + npr))

    xs = np.zeros((B, PXPAD), np.float32)
    xs[:, :npr * PPX] = x_pm[:, p0:p0 + npr, :].reshape(B, npr * PPX)
    # host-side transpose to [px_part 128, chunk, batch], bf16
    xs = np.ascontiguousarray(
        xs.reshape(B, NCHUNK, 128).transpose(2, 1, 0)
    ).astype(ml_dtypes.bfloat16).reshape(128, NCHUNK * B)

    wr = np.asarray(weight, np.float32).reshape(F, P, PPX)
    w_big = np.zeros((128, NMM * 32), np.float32)
    for i, (t, g, k, _, _) in enumerate(PLAN):
        for r in range(128):
            px = 128 * t + r
            p = px // PPX
            if p >= npr:
                continue
            pl = p - 8 * g
            if pl < 0 or pl // 2 != k:
                continue
            q = px % PPX
            w_big[r, 32 * i + (pl % 2) * 16:32 * i + (pl % 2) * 16 + F] = \
                wr[:, pids[p], q]
    w_big = w_big.astype(ml_dtypes.bfloat16)

    br = np.asarray(bias, np.float32).reshape(F, P)
    dr = np.asarray(dec_w, np.float32).reshape(OUT, F, P)
    b_st = np.zeros((128, 2), np.float32)
    d_st = np.zeros((128, 2 * OUT), np.float32)
    for p in range(npr):
        g, pl = p // 8, p % 8
        j = 16 * pl + np.arange(F)
        b_st[j, g] = br[:, pids[p]]
        d_st[j[:, None], g * OUT + np.arange(OUT)[None, :]] = dr[:, :, pids[p]].T
    d_st = d_st.astype(ml_dtypes.bfloat16)
    return {"x": xs, "w": w_big, "bias": b_st, "dec": d_st}


_cache = {}


def _get_nc():
    if "nc" not in _cache:
        nc = build_program()
        nc.finalize()
        _cache["nc"] = nc
    return _cache["nc"]


def make_in_maps(x, weight, bias, dec_w):
    x = np.asarray(x, np.float32)
    # patch-major pixel order: (b, ph, pw, k, l)
    x_pm = np.ascontiguousarray(
        x.reshape(B, 10, 28, 10, 28).transpose(0, 1, 3, 2, 4)
    ).reshape(B, P, PPX)
    return [stage_core(c, x_pm, weight, bias, dec_w) for c in range(NCORES)]


def combine(results, dec_b):
    acc = np.zeros((OUT, B), np.float32)
    for r in results:
        acc += r["out"]
    return acc.T + np.asarray(dec_b, np.float32)


def _install_ntff_hook():
    """Provide the missing antenv.axon_hooks module so trace=True works
    under axon (replicates trn_boot._ntff_profile_via_ctypes)."""
    import contextlib
    import ctypes
    import types

    if "antenv.axon_hooks" in sys.modules:
        return
    so_path = "/opt/axon/libaxon_pjrt.so"
    holder = {}
    mod = types.ModuleType("antenv.axon_hooks")
    mod.set_axon_ntff_profile_hook = lambda h: holder.__setitem__("h", h)
    mod.get_axon_ntff_profile_hook = lambda: holder.get("h")
    sys.modules["antenv.axon_hooks"] = mod
    try:
        import antenv
        antenv.axon_hooks = mod
    except ImportError:
        pass

    lib = ctypes.CDLL(so_path)
    if not hasattr(lib, "axon_start_nrt_profile"):
        return
    lib.axon_start_nrt_profile.argtypes = [
        ctypes.POINTER(ctypes.c_int64), ctypes.c_size_t]
    lib.axon_start_nrt_profile.restype = ctypes.c_int64
    lib.axon_stop_nrt_profile.argtypes = [ctypes.c_char_p]
    lib.axon_stop_nrt_profile.restype = ctypes.c_int64

    @contextlib.contextmanager
    def _hook(output_dir, device_ids):
        import jax
        jax.devices()
        if device_ids:
            ids = (ctypes.c_int64 * len(device_ids))(*device_ids)
            rc = lib.axon_start_nrt_profile(ids, len(device_ids))
        else:
            rc = lib.axon_start_nrt_profile(None, 0)
        if rc != 0:
            raise RuntimeError(f"axon_start_nrt_profile rc={rc}")
        try:
            yield
        finally:
            n = lib.axon_stop_nrt_profile(str(output_dir).encode())
            print(f"profile: {n} file(s) written to {output_dir}")

    mod.set_axon_ntff_profile_hook(_hook)


def run(x, weight, bias, dec_w, dec_b, trace=False):
    from concourse import bass_utils
    from concourse.bass_utils import run_bass_kernel_spmd

    if trace:
        _install_ntff_hook()
        bass_utils.upload_artifacts = lambda tmpdir: tmpdir

    nc = _get_nc()
    in_maps = make_in_maps(x, weight, bias, dec_w)
    r = run_bass_kernel_spmd(nc, in_maps, list(range(NCORES)), trace=trace)
    return combine(r.results, dec_b), r


def kernel(x, weight, bias, dec_w, dec_b):
    out, _ = run(x, weight, bias, dec_w, dec_b, trace=False)
    return out


# revision 18
# speedup vs baseline: 4.5512x; 1.0985x over previous
"""Trainium2 Bass kernel for nn_LCN (locally-connected network).

Computation:
  x: (512, 1, 280, 280) -> non-overlapping 28x28 patches (10x10 grid, P=100)
  y[b, f, p] = sum_q x[b, p, q] * w[f*100+p, q]    (q = k*28+l, 784 per patch)
  y = relu(y + bias[f*100+p]);  out = y_flat @ dec_w.T + dec_b  (j = f*100+p)

Sharding: patch-parallel. All 8 cores see all 512 images; core c owns 13
patches (cores 4-7 own 12 real + 1 zero patch so every core runs the same
program). Per core:
  - host stages x TRANSPOSED as xT [128 px, 80 chunks, 512 b] bf16
    (im2col + transpose + cast all on host; DMA reads are contiguous
    multi-KB runs per partition at full HBM bandwidth)
  - conv: one matmul per (128-px chunk, patch-pair window): lhsT = staged
    weight tile [128, 32], rhs = xT[:, t, :] [128, 512] -> PSUM [128, 512]
    accumulating per 8-patch group (partition j = 16*local_patch + f)
  - ACT: relu(psum + bias) -> y_sb
  - decoder: 2 accumulating matmuls lhsT=dec [K, 10] -> out [10, 512]
Host sums the 8 per-core partial decoder outputs and adds dec_b.
"""

import sys

import numpy as np

for _p in ("/opt/trn_rl_repo", "/opt/trn_rl_repo/concourse"):
    if _p not in sys.path:
        sys.path.insert(0, _p)

import concourse.bass as bass
import concourse.mybir as mybir
import concourse.tile as tile
from concourse import bacc

F32 = mybir.dt.float32
BF16 = mybir.dt.bfloat16

# Problem constants
B = 512
P = 100
F = 16
OUT = 10
PPX = 784            # pixels per patch (28*28)
NCORES = 8

NPAT = 13            # patches per core program (zero-padded on 12-patch cores)
NCHUNK = 80          # ceil(13*784/128)
PXPAD = NCHUNK * 128  # 10240
GROUPS = [(0, 8), (8, 5)]   # (first local patch, n patches incl virtual pad)
GROUP_CHUNKS = [(0, 49), (49, 31)]  # (first chunk, n chunks); 8*784 = 49*128
# small first split to warm up PE early, small last split to trim the tail
_SPLIT_SIZES = [4, 8, 12, 12, 12, 12, 14, 6]
DMA_SPLITS = []
_c = 0
for _s in _SPLIT_SIZES:
    DMA_SPLITS.append((_c, _s))
    _c += _s
assert _c == NCHUNK

# per-core real patch ranges (cores 0-3: 13 patches, cores 4-7: 12)
CORE_PSTART = [0, 13, 26, 39, 52, 64, 76, 88]
CORE_NPAT = [13, 13, 13, 13, 12, 12, 12, 12]


def conv_plan():
    """Static matmul plan: one entry per (chunk, patch-pair window)."""
    plan = []
    for t in range(NCHUNK):
        p0 = (128 * t) // PPX
        p1 = (128 * t + 127) // PPX
        g = p0 // 8
        pairs = sorted({(min(p, 8 * g + 9) - 8 * g) // 2 for p in (p0, p1)})
        for k in pairs:
            first = ((8 * g + 2 * k) * PPX) // 128
            last = min(((8 * g + 2 * k + 2) * PPX - 1) // 128, NCHUNK - 1)
            plan.append((t, g, k, t == first, t == last))
    return plan

PLAN = conv_plan()
NMM = len(PLAN)  # 85


def build_program():
    nc = bacc.Bacc("TRN2")
    x_d = nc.dram_tensor("x", [128, NCHUNK * B], BF16, kind="ExternalInput")
    # w tiles then decoder staged in one tensor / one DMA
    w_d = nc.dram_tensor("w", [128, NMM * 32 + 2 * OUT], BF16,
                         kind="ExternalInput")
    b_d = nc.dram_tensor("bias", [128, 2], F32, kind="ExternalInput")
    o_d = nc.dram_tensor("out", [OUT, B], F32, kind="ExternalOutput")

    with tile.TileContext(nc) as tc:
        with (
            tc.tile_pool(name="const", bufs=1) as constp,
            tc.tile_pool(name="yps", bufs=2, space="PSUM") as ypsp,
            tc.tile_pool(name="wps", bufs=1, space="PSUM") as wpsp,
            tc.tile_pool(name="ops", bufs=1, space="PSUM") as opsp,
        ):
            w_sb = constp.tile([128, NMM * 32 + 2 * OUT], BF16)
            nc.scalar.dma_start(out=w_sb[:], in_=w_d[:])
            dec_sb = w_sb[:, NMM * 32:]
            bias_sb = constp.tile([128, 2], F32)
            nc.scalar.dma_start(out=bias_sb[:], in_=b_d[:])

            xt = constp.tile([128, NCHUNK, B], BF16)
            for c0, n in DMA_SPLITS:
                nc.sync.dma_start(
                    out=xt[:, c0:c0 + n, :],
                    in_=x_d[:, c0 * B:(c0 + n) * B],
                )

            # PE clock warm-up during the DMA pipe-fill window: M=128 dummy
            # matmuls on an uninitialized scratch tile (no input dependency,
            # output never read)
            warm_sb = constp.tile([128, 128 + B], BF16)
            nc.gpsimd.memset(warm_sb[:], 0.0)
            warm_ps = wpsp.tile([128, B], F32)
            for _ in range(9):
                nc.tensor.matmul(
                    warm_ps[:],
                    warm_sb[:, 0:128],
                    warm_sb[:, 128:],
                    start=True,
                    stop=True,
                )

            y_sb = constp.tile([128, 2, B], BF16)
            ps = [ypsp.tile([128, B], F32, name=f"ps{g}") for g in range(2)]
            out_ps = opsp.tile([OUT, B], F32)

            rows = [16 * 8, 16 * 5]  # evacuated rows per group
            for g in range(2):
                gi = [e for e in enumerate(PLAN) if e[1][1] == g]
                for i, (t, _, k, st, sp) in gi:
                    nc.tensor.matmul(
                        ps[g][32 * k:32 * k + 32, :],
                        w_sb[:, 32 * i:32 * i + 32],
                        xt[:, t, :],
                        start=st,
                        stop=sp,
                        tile_position=(0, 32 * k),
                    )
                nc.scalar.activation(
                    out=y_sb[0:rows[g], g, :],
                    in_=ps[g][0:rows[g], :],
                    func=mybir.ActivationFunctionType.Relu,
                    bias=bias_sb[0:rows[g], g:g + 1],
                )
                nc.tensor.matmul(
                    out_ps[:],
                    dec_sb[0:rows[g], g * OUT:(g + 1) * OUT],
                    y_sb[0:rows[g], g, :],
                    start=(g == 0),
                    stop=(g == 1),
                )
            out_sb = constp.tile([OUT, B], F32)
            nc.vector.tensor_copy(out_sb[:], out_ps[:])
            nc.sync.dma_start(out=o_d[:], in_=out_sb[:])

    return nc


def stage_core(core, x_pm, weight, bias, dec_w):
    """Host-side staging for one core. x_pm: (B, 100, 784) float32."""
    import ml_dtypes

    p0 = CORE_PSTART[core]
    npr = CORE_NPAT[core]
    pids = list(range(p0, p0 + npr))

    xs = np.zeros((B, PXPAD), np.float32)
    xs[:, :npr * PPX] = x_pm[:, p0:p0 + npr, :].reshape(B, npr * PPX)
    # host-side transpose to [px_part 128, chunk, batch], bf16
    xs = np.ascontiguousarray(
        xs.reshape(B, NCHUNK, 128).transpose(2, 1, 0)
    ).astype(ml_dtypes.bfloat16).reshape(128, NCHUNK * B)

    wr = np.asarray(weight, np.float32).reshape(F, P, PPX)
    w_big = np.zeros((128, NMM * 32), np.float32)
    for i, (t, g, k, _, _) in enumerate(PLAN):
        for r in range(128):
            px = 128 * t + r
            p = px // PPX
            if p >= npr:
                continue
            pl = p - 8 * g
            if pl < 0 or pl // 2 != k:
                continue
            q = px % PPX
            w_big[r, 32 * i + (pl % 2) * 16:32 * i + (pl % 2) * 16 + F] = \
                wr[:, pids[p], q]

    br = np.asarray(bias, np.float32).reshape(F, P)
    dr = np.asarray(dec_w, np.float32).reshape(OUT, F, P)
    b_st = np.zeros((128, 2), np.float32)
    d_st = np.zeros((128, 2 * OUT), np.float32)
    for p in range(npr):
        g, pl = p // 8, p % 8
        j = 16 * pl + np.arange(F)
        b_st[j, g] = br[:, pids[p]]
        d_st[j[:, None], g * OUT + np.arange(OUT)[None, :]] = dr[:, :, pids[p]].T
    w_all = np.concatenate(
        [w_big, d_st], axis=1).astype(ml_dtypes.bfloat16)
    return {"x": xs, "w": w_all, "bias": b_st}


_cache = {}


def _get_nc():
    if "nc" not in _cache:
        nc = build_program()
        nc.finalize()
        _cache["nc"] = nc
    return _cache["nc"]


def make_in_maps(x, weight, bias, dec_w):
    x = np.asarray(x, np.float32)
    # patch-major pixel order: (b, ph, pw, k, l)
    x_pm = np.ascontiguousarray(
        x.reshape(B, 10, 28, 10, 28).transpose(0, 1, 3, 2, 4)
    ).reshape(B, P, PPX)
    return [stage_core(c, x_pm, weight, bias, dec_w) for c in range(NCORES)]


def combine(results, dec_b):
    acc = np.zeros((OUT, B), np.float32)
    for r in results:
        acc += r["out"]
    return acc.T + np.asarray(dec_b, np.float32)


def _install_ntff_hook():
    """Provide the missing antenv.axon_hooks module so trace=True works
    under axon (replicates trn_boot._ntff_profile_via_ctypes)."""
    import contextlib
    import ctypes
    import types

    if "antenv.axon_hooks" in sys.modules:
        return
    so_path = "/opt/axon/libaxon_pjrt.so"
    holder = {}
    mod = types.ModuleType("antenv.axon_hooks")
    mod.set_axon_ntff_profile_hook = lambda h: holder.__setitem__("h", h)
    mod.get_axon_ntff_profile_hook = lambda: holder.get("h")
    sys.modules["antenv.axon_hooks"] = mod
    try:
        import antenv
        antenv.axon_hooks = mod
    except ImportError:
        pass

    lib = ctypes.CDLL(so_path)
    if not hasattr(lib, "axon_start_nrt_profile"):
        return
    lib.axon_start_nrt_profile.argtypes = [
        ctypes.POINTER(ctypes.c_int64), ctypes.c_size_t]
    lib.axon_start_nrt_profile.restype = ctypes.c_int64
    lib.axon_stop_nrt_profile.argtypes = [ctypes.c_char_p]
    lib.axon_stop_nrt_profile.restype = ctypes.c_int64

    @contextlib.contextmanager
    def _hook(output_dir, device_ids):
        import jax
        jax.devices()
        if device_ids:
            ids = (ctypes.c_int64 * len(device_ids))(*device_ids)
            rc = lib.axon_start_nrt_profile(ids, len(device_ids))
        else:
            rc = lib.axon_start_nrt_profile(None, 0)
        if rc != 0:
            raise RuntimeError(f"axon_start_nrt_profile rc={rc}")
        try:
            yield
        finally:
            n = lib.axon_stop_nrt_profile(str(output_dir).encode())
            print(f"profile: {n} file(s) written to {output_dir}")

    mod.set_axon_ntff_profile_hook(_hook)


def run(x, weight, bias, dec_w, dec_b, trace=False):
    from concourse import bass_utils
    from concourse.bass_utils import run_bass_kernel_spmd

    if trace:
        _install_ntff_hook()
        bass_utils.upload_artifacts = lambda tmpdir: tmpdir

    nc = _get_nc()
    in_maps = make_in_maps(x, weight, bias, dec_w)
    r = run_bass_kernel_spmd(nc, in_maps, list(range(NCORES)), trace=trace)
    return combine(r.results, dec_b), r


def kernel(x, weight, bias, dec_w, dec_b):
    out, _ = run(x, weight, bias, dec_w, dec_b, trace=False)
    return out
